# revision 2
# baseline (speedup 1.0000x reference)
"""GQA attention (B=2, S=2048, D=2048, H=32, G=8, hd=64) on 8 TRN2 cores.

Sharding: core c owns (batch b=c//4, token block q0=512*(c%4)). K/V
projection is TOKEN-SHARDED: each core projects K/V only for its own 512
tokens, then a single packed AllGather (1.03 MB/core) within each 4-core
batch group assembles full-S K^T and augmented V. This removes the 4x
replicated K/V projection work (192 of 512 projection matmuls/core).
Everything downstream (attention, denominators, output projection) is
fully local to the core's 512 query tokens.

Layouts:
  - x^T own-tokens only [128, 16, 512]; K^T [128(2 groups), 4, 512-own]
    and V [tok-part, feat] projected on-chip into a combined kv tile
    [128, 4, 1032] (= kt 512 | vaug 520) that is gathered in one
    collective; V is stored augmented with a ones column per head so the
    PV matmul accumulates the softmax denominator in PSUM row 64 for
    free. 1/d via exp(-ln(d)), folded into o^T before the output
    projection.
  - scores come out transposed [k, q] so PV needs no on-chip transposes.
  - head pair (hA=8t+r, hB=8t+4+r) processed together: one [128,1024]
    PSUM scores tile (A|B), one exp() activation covers both heads.
"""

import sys

sys.path.insert(0, "/opt/trn_rl_repo")

import numpy as np
import ml_dtypes

import concourse.bass as bass
import concourse.tile as tile
from concourse import bacc, mybir
from concourse.bass_utils import run_bass_kernel_spmd

BF16 = ml_dtypes.bfloat16
B, S, D = 2, 2048, 2048
H, G, HD = 32, 8, 64
DC = D // 128  # 16 dim chunks
N_CORES = 8
TOK = 512  # own tokens per core

_CACHE = {}


def _build():
    f32 = mybir.dt.float32
    bf16 = mybir.dt.bfloat16
    nc = bacc.Bacc("TRN2", target_bir_lowering=False, debug=False, num_devices=N_CORES)

    xt = nc.dram_tensor("xt", [128, DC, TOK], bf16, kind="ExternalInput").ap()
    wq = nc.dram_tensor("wq", [128, DC, DC, 128], bf16, kind="ExternalInput").ap()
    wk = nc.dram_tensor("wk", [128, DC, 4, 128], bf16, kind="ExternalInput").ap()
    wv = nc.dram_tensor("wv", [128, DC, 512], bf16, kind="ExternalInput").ap()
    cosr = nc.dram_tensor("cosr", [128, TOK], bf16, kind="ExternalInput").ap()
    sinr = nc.dram_tensor("sinr", [128, TOK], bf16, kind="ExternalInput").ap()
    wo = nc.dram_tensor("wo", [128, DC, D], bf16, kind="ExternalInput").ap()
    out = nc.dram_tensor("out", [TOK, D], f32, kind="ExternalOutput").ap()

    Exp = mybir.ActivationFunctionType.Exp
    Ln = mybir.ActivationFunctionType.Ln
    swap_mask = [i ^ 1 for i in range(32)]
    scale = float(1.0 / np.sqrt(HD))

    from contextlib import ExitStack
    with tile.TileContext(nc) as tc, ExitStack() as ctx:
        consts = ctx.enter_context(tc.tile_pool(name="consts", bufs=1))
        wqp = ctx.enter_context(tc.tile_pool(name="wqp", bufs=2))
        io = ctx.enter_context(tc.tile_pool(name="io", bufs=2))
        work = ctx.enter_context(tc.tile_pool(name="work", bufs=3))
        outw = ctx.enter_context(tc.tile_pool(name="outw", bufs=2))
        # scores get their own 2x[128,1024] pool (4 banks); projection
        # accumulators + PV accumulators + outproj share a 4x[*,512] pool
        # (4 banks) so attention overlaps the projections.
        psum = ctx.enter_context(tc.tile_pool(name="psum", bufs=2, space="PSUM"))
        opsum = ctx.enter_context(tc.tile_pool(name="opsum", bufs=4, space="PSUM"))
        dram = ctx.enter_context(tc.tile_pool(name="dram", bufs=1, space="DRAM"))

        # ---- load inputs, ordered for the earliest possible first vproj
        # matmul: wv + x^T chunks feed it; wk/cos/sin follow
        wv_sb = consts.tile([128, DC, 512], bf16, tag="wv")
        nc.sync.dma_start(out=wv_sb[:], in_=wv[:])
        xt_sb = consts.tile([128, DC, TOK], bf16, tag="xt")
        nc.sync.dma_start(out=xt_sb[:, 0:4, :], in_=xt[:, 0:4, :])
        wk_sb = consts.tile([128, DC, 4, 128], bf16, tag="wk")
        nc.sync.dma_start(out=wk_sb[:], in_=wk[:])
        for i in range(1, 4):
            nc.sync.dma_start(out=xt_sb[:, 4 * i:4 * i + 4, :],
                              in_=xt[:, 4 * i:4 * i + 4, :])
        cos_sb = consts.tile([128, TOK], bf16, tag="cos")
        nc.sync.dma_start(out=cos_sb[:], in_=cosr[:])
        sin_sb = consts.tile([128, TOK], bf16, tag="sin")
        nc.sync.dma_start(out=sin_sb[:], in_=sinr[:])

        # combined own-token K^T|Vaug tile: [:, t, 0:512] = kt tile t (own
        # 512 tokens), [:, tb, 512:1032] = vaug for own token block tb
        kv_own = consts.tile([128, 4, 1032], bf16, tag="kv_own")
        # gathered full-S K^T and augmented V
        kt_sb = consts.tile([128, 4, S], bf16, tag="kt")
        vaug_sb = consts.tile([128, DC, 520], bf16, tag="vaug")
        qt_sb = consts.tile([128, DC, TOK], bf16, tag="wv")  # wv dead post-vproj
        # denominator staging: pair fc -> partition 32*(fc%4), free block fc//4
        # (DVE partition bases must be 32-aligned). memset(1) keeps the unused
        # rows finite through the Ln/Exp pass.
        dstage = consts.tile([97, 4096], f32, tag="dstage")
        nc.vector.memset(dstage[:], 1.0)
        ddram = dram.tile([4, 4096], f32, tag="dd", name="dd")

        # ones columns of augmented V (512+130t+64 for head A, +129 for B)
        for t in range(4):
            nc.vector.memset(kv_own[:, :, 512 + 130 * t + 64:512 + 130 * t + 65], 1.0)
            nc.vector.memset(kv_own[:, :, 512 + 130 * t + 129:512 + 130 * t + 130], 1.0)

        def rope(ap, cs, sn):
            sw = io.tile([128, 512], bf16, tag="rsw")
            nc.vector.stream_shuffle(sw, ap, swap_mask)
            nc.vector.tensor_mul(sw, sw, sn)
            tmp = io.tile([128, 512], bf16, tag="rtmp")
            nc.vector.tensor_mul(tmp, ap, cs)
            nc.vector.tensor_add(ap, sw, tmp)

        # ---- V projection (own tokens) into augmented layout
        for tb in range(4):
            ps = opsum.tile([128, 512], f32, tag="o")
            for c in range(DC):
                nc.tensor.matmul(
                    ps,
                    lhsT=xt_sb[:, c, tb * 128:(tb + 1) * 128],
                    rhs=wv_sb[:, c, :],
                    start=(c == 0), stop=(c == DC - 1),
                )
            for t in range(4):
                nc.vector.tensor_copy(
                    kv_own[:, tb, 512 + 130 * t:512 + 130 * t + 64],
                    ps[:, t * 128:t * 128 + 64])
                nc.vector.tensor_copy(
                    kv_own[:, tb, 512 + 130 * t + 65:512 + 130 * t + 129],
                    ps[:, t * 128 + 64:t * 128 + 128])

        # ---- K projection (own tokens) + rope
        for t in range(4):
            ps = opsum.tile([128, 512], f32, tag="o")
            for c in range(DC):
                nc.tensor.matmul(
                    ps,
                    lhsT=wk_sb[:, c, t, :],
                    rhs=xt_sb[:, c, :],
                    start=(c == 0), stop=(c == DC - 1),
                )
            nc.vector.tensor_copy(kv_own[:, t, 0:512], ps)
            rope(kv_own[:, t, 0:512], cos_sb[:], sin_sb[:])

        # ---- gather K/V across the 4-core batch group (one collective)
        kvd_in = dram.tile([128, 4, 1032], bf16, tag="kvi", name="kvi")
        kvd_out = dram.tile([4, 128, 4, 1032], bf16, tag="kvo", name="kvo")
        nc.sync.dma_start(out=kvd_in[:], in_=kv_own[:])
        nc.gpsimd.collective_compute(
            "AllGather",
            mybir.AluOpType.bypass,
            replica_groups=[[0, 1, 2, 3], [4, 5, 6, 7]],
            ins=[kvd_in[:]],
            outs=[kvd_out[:]],
        )
        for j in range(4):
            nc.sync.dma_start(out=kt_sb[:, :, 512 * j:512 * j + 512],
                              in_=kvd_out[j, :, :, 0:512])
            nc.sync.dma_start(out=vaug_sb[:, 4 * j:4 * j + 4, :],
                              in_=kvd_out[j, :, :, 512:1032])

        # ot reuses the wk slot (wk dead after kproj)
        ot_sb = consts.tile([128, DC, TOK], bf16, tag="wk")

        def recip_quarter(t):
            # 1/d for pairs fc in [4t, 4t+4): dstage free cols [1024t, 1024t+1024)
            hs = slice(1024 * t, 1024 * (t + 1))
            nc.scalar.activation(dstage[:, hs], dstage[:, hs], Ln)
            nc.scalar.activation(dstage[:, hs], dstage[:, hs], Exp, scale=-1.0)
            for rr in range(4):
                nc.sync.dma_start(out=ddram[rr:rr + 1, hs],
                                  in_=dstage[32 * rr:32 * rr + 1, hs])
            for fc in range(4 * t, 4 * t + 4):
                dp = fc % 4
                df = (fc // 4) * 1024
                r2 = io.tile([128, TOK], bf16, tag="r2")
                nc.gpsimd.dma_start(
                    out=r2[0:64, :],
                    in_=ddram[dp:dp + 1, df:df + 512].partition_broadcast(64))
                nc.gpsimd.dma_start(
                    out=r2[64:128, :],
                    in_=ddram[dp:dp + 1, df + 512:df + 1024].partition_broadcast(64))
                nc.vector.tensor_mul(ot_sb[:, fc, :], ot_sb[:, fc, :], r2)

        # ---- Q projection round: 4 fc chunks (one attention round's worth)
        def proj_round(t):
            for ssl in range(4):
                fc = 4 * t + ssl
                wq_t = wqp.tile([128, DC, 128], bf16, tag="wq")
                nc.sync.dma_start(out=wq_t[:], in_=wq[:, fc, :, :])
                ps = opsum.tile([128, 512], f32, tag="o")
                for c in range(DC):
                    nc.tensor.matmul(
                        ps,
                        lhsT=wq_t[:, c, :],
                        rhs=xt_sb[:, c, :],
                        start=(c == 0), stop=(c == DC - 1),
                    )
                nc.vector.tensor_copy(qt_sb[:, fc, :], ps)
                rope(qt_sb[:, fc, :], cos_sb[:], sin_sb[:])

        def attn_round(t, r0, r1):
            # attention pairs of this t: heads (8t+r | 8t+4+r), fc = 4t+r
            for r in range(r0, r1):
                fc = 4 * t + r
                oA = opsum.tile([65, 512], f32, tag="o")
                oB = opsum.tile([65, 512], f32, tag="o")
                for kb in range(DC):
                    ksl = slice(kb * 128, (kb + 1) * 128)
                    s = psum.tile([128, 1024], f32, tag="s")
                    nc.tensor.matmul(
                        s[:, 0:512], lhsT=kt_sb[0:64, t, ksl],
                        rhs=qt_sb[0:64, fc, :],
                        start=True, stop=True, tile_position=(0, 0),
                    )
                    nc.tensor.matmul(
                        s[:, 512:1024], lhsT=kt_sb[64:128, t, ksl],
                        rhs=qt_sb[64:128, fc, :],
                        start=True, stop=True, tile_position=(64, 0),
                    )
                    p = work.tile([128, 1024], bf16, tag="p")
                    nc.scalar.activation(p, s, Exp, scale=scale)
                    nc.tensor.matmul(
                        oA, lhsT=vaug_sb[:, kb, 130 * t:130 * t + 65],
                        rhs=p[:, 0:512],
                        start=(kb == 0), stop=(kb == DC - 1),
                    )
                    nc.tensor.matmul(
                        oB, lhsT=vaug_sb[:, kb, 130 * t + 65:130 * t + 130],
                        rhs=p[:, 512:1024],
                        start=(kb == 0), stop=(kb == DC - 1),
                    )
                nc.vector.tensor_copy(ot_sb[0:64, fc, :], oA[0:64, :])
                nc.vector.tensor_copy(ot_sb[64:128, fc, :], oB[0:64, :])
                dp = 32 * (fc % 4)
                df = (fc // 4) * 1024
                nc.vector.tensor_copy(
                    dstage[dp:dp + 1, df:df + 512], oA[64:65, :])
                nc.vector.tensor_copy(
                    dstage[dp:dp + 1, df + 512:df + 1024], oB[64:65, :])


        proj_round(0)
        attn_round(0, 0, 2)
        proj_round(1)
        attn_round(0, 2, 4)
        recip_quarter(0)
        proj_round(2)
        attn_round(1, 0, 4)
        recip_quarter(1)
        proj_round(3)
        # wo gets its own slot (xt too small to reuse now)
        wo_sb = consts.tile([128, DC, D], bf16, tag="wo")
        for i in range(4):
            nc.sync.dma_start(out=wo_sb[:, 4 * i:4 * i + 4, :],
                              in_=wo[:, 4 * i:4 * i + 4, :])
        attn_round(2, 0, 4)
        recip_quarter(2)
        attn_round(3, 0, 3)
        # last quarter's reciprocals for pairs r=0..2 early (rows 0/32/64),
        # so only row 96 remains after the final pair
        hs3 = slice(3072, 4096)
        nc.scalar.activation(dstage[0:65, hs3], dstage[0:65, hs3], Ln)
        nc.scalar.activation(dstage[0:65, hs3], dstage[0:65, hs3], Exp,
                             scale=-1.0)
        for rr in range(3):
            nc.sync.dma_start(out=ddram[rr:rr + 1, hs3],
                              in_=dstage[32 * rr:32 * rr + 1, hs3])
        for fc in (12, 13, 14):
            df = (fc // 4) * 1024
            r2 = io.tile([128, TOK], bf16, tag="r2")
            nc.gpsimd.dma_start(
                out=r2[0:64, :],
                in_=ddram[fc % 4:fc % 4 + 1, df:df + 512].partition_broadcast(64))
            nc.gpsimd.dma_start(
                out=r2[64:128, :],
                in_=ddram[fc % 4:fc % 4 + 1, df + 512:df + 1024].partition_broadcast(64))
            nc.vector.tensor_mul(ot_sb[:, fc, :], ot_sb[:, fc, :], r2)
        attn_round(3, 3, 4)
        # warm-keepers: harmless matmuls bridge the final reciprocal chain so
        # the PE HAM clock stays at 8/8 and the output projection starts warm
        for _ in range(12):
            sdum = psum.tile([128, 1024], f32, tag="s")
            nc.tensor.matmul(
                sdum[0:65, 0:512], lhsT=vaug_sb[:, 0, 0:65],
                rhs=qt_sb[:, 0, :], start=True, stop=True,
            )
        nc.scalar.activation(dstage[96:97, hs3], dstage[96:97, hs3], Ln)
        nc.scalar.activation(dstage[96:97, hs3], dstage[96:97, hs3], Exp,
                             scale=-1.0)
        nc.sync.dma_start(out=ddram[3:4, hs3], in_=dstage[96:97, hs3])
        r2 = io.tile([128, TOK], bf16, tag="r2")
        nc.gpsimd.dma_start(
            out=r2[0:64, :],
            in_=ddram[3:4, 3072:3584].partition_broadcast(64))
        nc.gpsimd.dma_start(
            out=r2[64:128, :],
            in_=ddram[3:4, 3584:4096].partition_broadcast(64))
        nc.vector.tensor_mul(ot_sb[:, 15, :], ot_sb[:, 15, :], r2)

        # ---- output projection: out[tok, D] = o_norm @ wo.T
        for tb2 in range(4):
            tsl = slice(tb2 * 128, (tb2 + 1) * 128)
            for dc in range(4):
                dsl = slice(dc * 512, (dc + 1) * 512)
                ps = opsum.tile([128, 512], f32, tag="o")
                for fc in range(DC):
                    nc.tensor.matmul(
                        ps,
                        lhsT=ot_sb[:, fc, tsl],
                        rhs=wo_sb[:, fc, dsl],
                        start=(fc == 0), stop=(fc == DC - 1),
                    )
                osb = outw.tile([128, 512], f32, tag="osb")
                nc.vector.tensor_copy(osb, ps)
                nc.sync.dma_start(out=out[tsl, dsl], in_=osb)

    nc.compile()
    return nc


def _prep_shared(freqs_cos, freqs_sin, wqkv, wo):
    """Weight/table prep shared by all cores."""
    cs = np.asarray(freqs_cos)[:, 0, :]  # [S, 64] (already repeat-2 layout)
    sn = np.asarray(freqs_sin)[:, 0, :]
    cos_h = np.empty((128, S), np.float32)
    sin_h = np.empty((128, S), np.float32)
    for p in range(128):
        cos_h[p] = cs[:, p % 64]
        sin_h[p] = sn[:, p % 64] * (-1.0 if p % 2 == 0 else 1.0)

    # Q rows permuted: fc = 4t+r -> [head 8t+r | head 8t+4+r]
    qrows = []
    for t in range(4):
        for r in range(4):
            for h in (8 * t + r, 8 * t + 4 + r):
                qrows.extend(range(h * HD, (h + 1) * HD))
    wq_t = np.ascontiguousarray(wqkv[qrows, :].T)  # [D, 2048]
    wq_h = np.ascontiguousarray(
        wq_t.reshape(DC, 128, DC, 128).transpose(1, 2, 0, 3)).astype(BF16)

    # K rows: tile t holds groups (2t | 2t+1)
    krows = []
    for t in range(4):
        for g in (2 * t, 2 * t + 1):
            krows.extend(range(H * HD + g * HD, H * HD + (g + 1) * HD))
    wk_t = np.ascontiguousarray(wqkv[krows, :].T)  # [D, 512]
    wk_h = np.ascontiguousarray(
        wk_t.reshape(DC, 128, 4, 128).transpose(1, 0, 2, 3)).astype(BF16)

    # V rows natural group order (cols t*128 : A 64 | B 64)
    vrows = list(range((H + G) * HD, (H + 2 * G) * HD))
    wv_t = np.ascontiguousarray(wqkv[vrows, :].T)  # [D, 512]
    wv_h = np.ascontiguousarray(
        wv_t.reshape(DC, 128, 512).transpose(1, 0, 2)).astype(BF16)

    # wo rhs: wo_h[p, fc, dcol] = wo[dcol, feat(fc, p)]
    feat = np.empty(D, np.int64)
    for fc in range(DC):
        t, r = divmod(fc, 4)
        for p in range(128):
            h = 8 * t + r + (4 if p >= 64 else 0)
            feat[fc * 128 + p] = h * HD + (p % 64)
    wo_h = np.ascontiguousarray(
        np.asarray(wo)[:, feat].T.reshape(DC, 128, D).transpose(1, 0, 2)
    ).astype(BF16)
    return cos_h, sin_h, wq_h, wk_h, wv_h, wo_h


def _prep_inputs(x, freqs_cos, freqs_sin, wqkv, wo):
    cos_h, sin_h, wq_h, wk_h, wv_h, wo_h = _prep_shared(
        freqs_cos, freqs_sin, wqkv, wo)
    x = np.asarray(x)
    ins = []
    for c in range(N_CORES):
        b, t4 = divmod(c, 4)
        q0 = t4 * TOK
        sl = slice(q0, q0 + TOK)
        xt_h = np.ascontiguousarray(
            x[b].T[:, sl].reshape(DC, 128, TOK).transpose(1, 0, 2)).astype(BF16)
        ins.append({
            "xt": xt_h,
            "wq": wq_h, "wk": wk_h, "wv": wv_h, "wo": wo_h,
            "cosr": np.ascontiguousarray(cos_h[:, sl]).astype(BF16),
            "sinr": np.ascontiguousarray(sin_h[:, sl]).astype(BF16),
        })
    return ins


TRACE = False


def kernel(x, freqs_cos, freqs_sin, wqkv, wo):
    if "nc" not in _CACHE:
        _CACHE["nc"] = _build()
    nc = _CACHE["nc"]
    ins = _prep_inputs(x, freqs_cos, freqs_sin, wqkv, wo)
    res = run_bass_kernel_spmd(nc, ins, list(range(N_CORES)), trace=TRACE)
    _CACHE["res"] = res
    out = np.empty((B, S, D), np.float32)
    for c in range(N_CORES):
        b, t4 = divmod(c, 4)
        out[b, t4 * TOK:(t4 + 1) * TOK, :] = res.results[c]["out"]
    return out


if __name__ == "__main__":
    rng = np.random.default_rng(0)
    x = rng.normal(size=(B, S, D)).astype(np.float32)
    fc_ = rng.random(size=(S, 1, HD)).astype(np.float32)
    fs_ = rng.random(size=(S, 1, HD)).astype(np.float32)
    wq_ = rng.normal(size=(3072, D)).astype(np.float32) * 0.02
    wo_ = rng.normal(size=(D, D)).astype(np.float32) * 0.02
    o = kernel(x, fc_, fs_, wq_, wo_)
    print(o.shape, o.dtype)


# revision 6
# speedup vs baseline: 1.0778x; 1.0778x over previous
"""GQA attention (B=2, S=2048, D=2048, H=32, G=8, hd=64) on 8 TRN2 cores.

Sharding: core c owns (batch b=c//4, token block q0=512*(c%4)). Hybrid
K/V distribution tuned around the ~60-100us fixed AllGather latency on
this runtime:
  - KV tiles t=0,1 (groups 0-3): REPLICATED compute over all S tokens
    (x^T streamed from HBM in two passes) so attention round 0 starts
    ~70us in, long before any collective lands.
  - KV tiles t=2,3 (groups 4-7): TOKEN-SHARDED — each core projects only
    its own 512 tokens, one small AllGather (0.52 MB) per 4-core batch
    group assembles them. Kicked at ~22us, needed at ~216us (round 2).
Attention, softmax denominators (augmented-V ones column), and the
output projection are fully local to the core's 512 query tokens.

Engine budget: ACT is the critical path (256 exp instrs ~294us); the
reciprocal 1/d runs on DVE (reciprocal_approx_fast) so ACT does pure exp
with a single table load. PSUM: scores 2x[128,1024] (4 banks) + chain
accumulators 2x[128,512] (2) + PV accumulator [65,1024] (2, both heads
side by side; row 64 = denominators). Output projection streams wo and
reuses all 8 banks as 8 held accumulators.
"""

import sys

sys.path.insert(0, "/opt/trn_rl_repo")

import numpy as np
import ml_dtypes

import concourse.bass as bass
import concourse.tile as tile
from concourse import bacc, mybir
from concourse.bass_utils import run_bass_kernel_spmd

BF16 = ml_dtypes.bfloat16
B, S, D = 2, 2048, 2048
H, G, HD = 32, 8, 64
DC = D // 128  # 16 dim chunks
N_CORES = 8
TOK = 512  # own tokens per core

_CACHE = {}


def _build():
    f32 = mybir.dt.float32
    bf16 = mybir.dt.bfloat16
    nc = bacc.Bacc("TRN2", target_bir_lowering=False, debug=False, num_devices=N_CORES)

    xt = nc.dram_tensor("xt", [128, DC, TOK], bf16, kind="ExternalInput").ap()
    xtf = nc.dram_tensor("xtf", [128, DC, S], bf16, kind="ExternalInput").ap()
    wq = nc.dram_tensor("wq", [128, DC, DC, 128], bf16, kind="ExternalInput").ap()
    wk = nc.dram_tensor("wk", [128, DC, 4, 128], bf16, kind="ExternalInput").ap()
    wv = nc.dram_tensor("wv", [128, DC, 512], bf16, kind="ExternalInput").ap()
    coso = nc.dram_tensor("coso", [128, TOK], bf16, kind="ExternalInput").ap()
    sino = nc.dram_tensor("sino", [128, TOK], bf16, kind="ExternalInput").ap()
    cosf = nc.dram_tensor("cosf", [128, S], bf16, kind="ExternalInput").ap()
    sinf = nc.dram_tensor("sinf", [128, S], bf16, kind="ExternalInput").ap()
    wo = nc.dram_tensor("wo", [128, DC, D], bf16, kind="ExternalInput").ap()
    out = nc.dram_tensor("out", [TOK, D], f32, kind="ExternalOutput").ap()

    Exp = mybir.ActivationFunctionType.Exp
    swap_mask = [i ^ 1 for i in range(32)]
    scale = float(1.0 / np.sqrt(HD))

    from contextlib import ExitStack
    with tile.TileContext(nc) as tc, ExitStack() as ctx:
        consts = ctx.enter_context(tc.tile_pool(name="consts", bufs=1))
        xtfp = ctx.enter_context(tc.tile_pool(name="xtfp", bufs=2))
        wqp = ctx.enter_context(tc.tile_pool(name="wqp", bufs=2))
        wop = ctx.enter_context(tc.tile_pool(name="wop", bufs=2))
        io = ctx.enter_context(tc.tile_pool(name="io", bufs=2))
        work = ctx.enter_context(tc.tile_pool(name="work", bufs=3))
        outw = ctx.enter_context(tc.tile_pool(name="outw", bufs=2))
        # PSUM: scores 2x[128,1024]=4 banks, chain accums 2x[128,512]=2,
        # PV accumulator 1x[65,1024]=2 (A|B side by side, row 64 = denom)
        psum = ctx.enter_context(tc.tile_pool(name="psum", bufs=2, space="PSUM"))
        opsum = ctx.enter_context(tc.tile_pool(name="opsum", bufs=2, space="PSUM"))
        apsum = ctx.enter_context(tc.tile_pool(name="apsum", bufs=1, space="PSUM"))
        dram = ctx.enter_context(tc.tile_pool(name="dram", bufs=1, space="DRAM"))

        # ---- input loads (sync queue; order = execution order)
        wk_sb = consts.tile([128, DC, 4, 128], bf16, tag="wk")
        nc.sync.dma_start(out=wk_sb[:], in_=wk[:])
        xt_sb = consts.tile([128, DC, TOK], bf16, tag="xt")
        nc.sync.dma_start(out=xt_sb[:], in_=xt[:])
        wv_sb = consts.tile([128, DC, 512], bf16, tag="wv")
        nc.sync.dma_start(out=wv_sb[:], in_=wv[:])
        cos_sb = consts.tile([128, TOK], bf16, tag="cos")
        nc.sync.dma_start(out=cos_sb[:], in_=coso[:])
        sin_sb = consts.tile([128, TOK], bf16, tag="sin")
        nc.sync.dma_start(out=sin_sb[:], in_=sino[:])
        cosf_sb = consts.tile([128, S], bf16, tag="cosf")
        nc.sync.dma_start(out=cosf_sb[:], in_=cosf[:])
        sinf_sb = consts.tile([128, S], bf16, tag="sinf")
        nc.sync.dma_start(out=sinf_sb[:], in_=sinf[:])

        # preheat the exp table set so the one ACT_TABLE_LOAD lands at ~5us
        preheat = io.tile([1, 64], bf16, tag="pre")
        nc.vector.memset(preheat[:], 0.0)
        nc.scalar.activation(preheat[:], preheat[:], Exp)

        # own-token K/V for tiles t=2,3 packed for the gather:
        # [:, 0:512]=kt2, [:, 512:1024]=kt3, [:, 1024+260*tb+...]=vaug own
        kv_own = consts.tile([128, 2064], bf16, tag="kv")
        kt_sb = consts.tile([128, 4, S], bf16, tag="kt")
        vaug_sb = consts.tile([128, DC, 520], bf16, tag="vaug")
        dstage = consts.tile([97, 4096], f32, tag="dstage")
        nc.vector.memset(dstage[:], 1.0)
        ddram = dram.tile([4, 4096], f32, tag="dd", name="dd")

        # ones columns: local region (tiles t=0,1 -> cols 0:260 of vaug_sb)
        for col in (64, 129, 194, 259):
            nc.vector.memset(vaug_sb[:, :, col:col + 1], 1.0)
        # ones columns: own/gathered region (t=2,3 inside kv_own)
        for tb in range(4):
            for col in (64, 129, 194, 259):
                c0 = 1024 + 260 * tb + col
                nc.vector.memset(kv_own[:, c0:c0 + 1], 1.0)

        def rope(ap, cs, sn):
            sw = io.tile([128, 512], bf16, tag="rsw")
            nc.vector.stream_shuffle(sw, ap, swap_mask)
            nc.vector.tensor_mul(sw, sw, sn)
            tmp = io.tile([128, 512], bf16, tag="rtmp")
            nc.vector.tensor_mul(tmp, ap, cs)
            nc.vector.tensor_add(ap, sw, tmp)

        # ---- own-token K (t=2,3) + rope
        for t in (2, 3):
            ps = opsum.tile([128, 512], f32, tag="o")
            for c in range(DC):
                nc.tensor.matmul(
                    ps, lhsT=wk_sb[:, c, t, :], rhs=xt_sb[:, c, :],
                    start=(c == 0), stop=(c == DC - 1),
                )
            dst = kv_own[:, 512 * (t - 2):512 * (t - 1)]
            nc.vector.tensor_copy(dst, ps)
            rope(dst, cos_sb[:], sin_sb[:])

        # ---- own-token V (feats 256:512 = tiles t=2,3), augmented layout
        for tb in range(4):
            ps = opsum.tile([128, 512], f32, tag="o")
            for c in range(DC):
                nc.tensor.matmul(
                    ps[:, 0:256],
                    lhsT=xt_sb[:, c, tb * 128:(tb + 1) * 128],
                    rhs=wv_sb[:, c, 256:512],
                    start=(c == 0), stop=(c == DC - 1),
                )
            for tt in range(2):
                base = 1024 + 260 * tb + 130 * tt
                nc.vector.tensor_copy(
                    kv_own[:, base:base + 64], ps[:, 128 * tt:128 * tt + 64])
                nc.vector.tensor_copy(
                    kv_own[:, base + 65:base + 129],
                    ps[:, 128 * tt + 64:128 * tt + 128])

        # ---- gather K/V tiles t=2,3 across the 4-core batch group.
        # All collective-side traffic rides the gpsimd queue so the sync
        # queue's streaming DMAs never sit behind the ~60-100us collective.
        kvd_in = dram.tile([128, 2064], bf16, tag="kvi", name="kvi")
        kvd_out = dram.tile([4, 128, 2064], bf16, tag="kvo", name="kvo")
        nc.gpsimd.dma_start(out=kvd_in[:], in_=kv_own[:])
        nc.gpsimd.collective_compute(
            "AllGather",
            mybir.AluOpType.bypass,
            replica_groups=[[0, 1, 2, 3], [4, 5, 6, 7]],
            ins=[kvd_in[:]],
            outs=[kvd_out[:]],
        )
        for j in range(4):
            nc.gpsimd.dma_start(out=kt_sb[:, 2:4, 512 * j:512 * j + 512],
                                in_=kvd_out[j, :, 0:1024])
            nc.gpsimd.dma_start(out=vaug_sb[:, 4 * j:4 * j + 4, 260:520],
                                in_=kvd_out[j, :, 1024:2064])

        # ot reuses the kv_own slot (dead once kvd_in is written)
        ot_sb = consts.tile([128, DC, TOK], bf16, tag="kv")
        # qt reuses the wv slot (wv dead after pass A's V chains)
        # -- allocated after pass A below.

        def passA_ssl(ssl):
            """K tile 0 + V feats 0:256 (tiles t=0,1) for 512 tokens."""
            xf = xtfp.tile([128, DC, 512], bf16, tag="xf")
            nc.sync.dma_start(out=xf[:], in_=xtf[:, :, 512 * ssl:512 * (ssl + 1)])
            sl = slice(512 * ssl, 512 * (ssl + 1))
            ps = opsum.tile([128, 512], f32, tag="o")
            for c in range(DC):
                nc.tensor.matmul(
                    ps, lhsT=wk_sb[:, c, 0, :], rhs=xf[:, c, :],
                    start=(c == 0), stop=(c == DC - 1),
                )
            nc.vector.tensor_copy(kt_sb[:, 0, sl], ps)
            rope(kt_sb[:, 0, sl], cosf_sb[:, sl], sinf_sb[:, sl])
            for tbl in range(4):
                tb = 4 * ssl + tbl
                ps = opsum.tile([128, 512], f32, tag="o")
                for c in range(DC):
                    nc.tensor.matmul(
                        ps[:, 0:256],
                        lhsT=xf[:, c, tbl * 128:(tbl + 1) * 128],
                        rhs=wv_sb[:, c, 0:256],
                        start=(c == 0), stop=(c == DC - 1),
                    )
                for tt in range(2):
                    base = 130 * tt
                    nc.vector.tensor_copy(
                        vaug_sb[:, tb, base:base + 64],
                        ps[:, 128 * tt:128 * tt + 64])
                    nc.vector.tensor_copy(
                        vaug_sb[:, tb, base + 65:base + 129],
                        ps[:, 128 * tt + 64:128 * tt + 128])

        def passB_ssl(ssl):
            """K tile 1 for 512 tokens."""
            xf = xtfp.tile([128, DC, 512], bf16, tag="xf")
            nc.sync.dma_start(out=xf[:], in_=xtf[:, :, 512 * ssl:512 * (ssl + 1)])
            sl = slice(512 * ssl, 512 * (ssl + 1))
            ps = opsum.tile([128, 512], f32, tag="o")
            for c in range(DC):
                nc.tensor.matmul(
                    ps, lhsT=wk_sb[:, c, 1, :], rhs=xf[:, c, :],
                    start=(c == 0), stop=(c == DC - 1),
                )
            nc.vector.tensor_copy(kt_sb[:, 1, sl], ps)
            rope(kt_sb[:, 1, sl], cosf_sb[:, sl], sinf_sb[:, sl])

        def q_chain(fc, qt_sb):
            wq_t = wqp.tile([128, DC, 128], bf16, tag="wq")
            nc.sync.dma_start(out=wq_t[:], in_=wq[:, fc, :, :])
            ps = opsum.tile([128, 512], f32, tag="o")
            for c in range(DC):
                nc.tensor.matmul(
                    ps, lhsT=wq_t[:, c, :], rhs=xt_sb[:, c, :],
                    start=(c == 0), stop=(c == DC - 1),
                )
            nc.vector.tensor_copy(qt_sb[:, fc, :], ps)
            rope(qt_sb[:, fc, :], cos_sb[:], sin_sb[:])

        def attn_kb(t, fc, kb0, kb1, po, qt_sb):
            """Pair (head 8t+r | 8t+4+r), key blocks [kb0,kb1)."""
            for kb in range(kb0, kb1):
                ksl = slice(kb * 128, (kb + 1) * 128)
                s = psum.tile([128, 1024], f32, tag="s")
                nc.tensor.matmul(
                    s[:, 0:512], lhsT=kt_sb[0:64, t, ksl],
                    rhs=qt_sb[0:64, fc, :],
                    start=True, stop=True, tile_position=(0, 0),
                )
                nc.tensor.matmul(
                    s[:, 512:1024], lhsT=kt_sb[64:128, t, ksl],
                    rhs=qt_sb[64:128, fc, :],
                    start=True, stop=True, tile_position=(64, 0),
                )
                p = work.tile([128, 1024], bf16, tag="p")
                nc.scalar.activation(p, s, Exp, scale=scale)
                nc.tensor.matmul(
                    po[:, 0:512], lhsT=vaug_sb[:, kb, 130 * t:130 * t + 65],
                    rhs=p[:, 0:512],
                    start=(kb == 0), stop=(kb == DC - 1),
                )
                nc.tensor.matmul(
                    po[:, 512:1024], lhsT=vaug_sb[:, kb, 130 * t + 65:130 * t + 130],
                    rhs=p[:, 512:1024],
                    start=(kb == 0), stop=(kb == DC - 1),
                )

        def attn_finish(fc, po):
            nc.vector.tensor_copy(ot_sb[0:64, fc, :], po[0:64, 0:512])
            nc.vector.tensor_copy(ot_sb[64:128, fc, :], po[0:64, 512:1024])
            dp = 32 * (fc % 4)
            df = (fc // 4) * 1024
            nc.vector.tensor_copy(dstage[dp:dp + 1, df:df + 512],
                                  po[64:65, 0:512])
            nc.vector.tensor_copy(dstage[dp:dp + 1, df + 512:df + 1024],
                                  po[64:65, 512:1024])

        def attn_pair(t, r, qt_sb, interleave=()):
            """One head pair with optional callbacks between kb chunks.
            interleave: list of (after_kb, fn) run between chunks."""
            fc = 4 * t + r
            po = apsum.tile([65, 1024], f32, tag="po")
            points = sorted(set(k for k, _ in interleave))
            cuts = [0] + points + [DC]
            for i in range(len(cuts) - 1):
                if i > 0:
                    for k, fn in interleave:
                        if k == cuts[i]:
                            fn()
                attn_kb(t, fc, cuts[i], cuts[i + 1], po, qt_sb)
            attn_finish(fc, po)

        def recip_quarter(t):
            # 1/d on DVE (in-place), then broadcast via DRAM bounce rows
            hs = slice(1024 * t, 1024 * (t + 1))
            nc.vector.reciprocal_approx_fast(out=dstage[:, hs], in_=dstage[:, hs])
            for rr in range(4):
                nc.sync.dma_start(out=ddram[rr:rr + 1, hs],
                                  in_=dstage[32 * rr:32 * rr + 1, hs])
            for fc in range(4 * t, 4 * t + 4):
                dp = fc % 4
                df = (fc // 4) * 1024
                r2 = io.tile([128, TOK], bf16, tag="r2")
                nc.gpsimd.dma_start(
                    out=r2[0:64, :],
                    in_=ddram[dp:dp + 1, df:df + 512].partition_broadcast(64))
                nc.gpsimd.dma_start(
                    out=r2[64:128, :],
                    in_=ddram[dp:dp + 1, df + 512:df + 1024].partition_broadcast(64))
                nc.vector.tensor_mul(ot_sb[:, fc, :], ot_sb[:, fc, :], r2)

        # ================= schedule =================
        passA_ssl(0)
        qt_sb = consts.tile([128, DC, TOK], bf16, tag="qt")
        q_chain(0, qt_sb)
        # round 0 pair 0 rides along pass A: kb chunk (4*ssl) unlocks as
        # each ssl's K/V land
        po0 = apsum.tile([65, 1024], f32, tag="po")
        attn_kb(0, 0, 0, 4, po0, qt_sb)
        passA_ssl(1)
        q_chain(1, qt_sb)
        attn_kb(0, 0, 4, 8, po0, qt_sb)
        passA_ssl(2)
        q_chain(2, qt_sb)
        attn_kb(0, 0, 8, 12, po0, qt_sb)
        passA_ssl(3)
        q_chain(3, qt_sb)
        attn_kb(0, 0, 12, 16, po0, qt_sb)
        attn_finish(0, po0)

        attn_pair(0, 1, qt_sb, interleave=((8, lambda: passB_ssl(0)),))
        attn_pair(0, 2, qt_sb, interleave=(
            (4, lambda: passB_ssl(1)), (12, lambda: passB_ssl(2))))
        attn_pair(0, 3, qt_sb, interleave=(
            (4, lambda: passB_ssl(3)), (12, lambda: q_chain(4, qt_sb))))
        recip_quarter(0)

        attn_pair(1, 0, qt_sb, interleave=((8, lambda: q_chain(5, qt_sb)),))
        attn_pair(1, 1, qt_sb, interleave=((8, lambda: q_chain(6, qt_sb)),))
        attn_pair(1, 2, qt_sb, interleave=((8, lambda: q_chain(7, qt_sb)),))
        attn_pair(1, 3, qt_sb, interleave=((8, lambda: q_chain(8, qt_sb)),))
        recip_quarter(1)

        attn_pair(2, 0, qt_sb, interleave=((8, lambda: q_chain(9, qt_sb)),))
        attn_pair(2, 1, qt_sb, interleave=((8, lambda: q_chain(10, qt_sb)),))
        attn_pair(2, 2, qt_sb, interleave=((8, lambda: q_chain(11, qt_sb)),))
        attn_pair(2, 3, qt_sb, interleave=((8, lambda: q_chain(12, qt_sb)),))
        recip_quarter(2)

        attn_pair(3, 0, qt_sb, interleave=((8, lambda: q_chain(13, qt_sb)),))
        attn_pair(3, 1, qt_sb, interleave=((8, lambda: q_chain(14, qt_sb)),))
        attn_pair(3, 2, qt_sb, interleave=((8, lambda: q_chain(15, qt_sb)),))
        attn_pair(3, 3, qt_sb)
        # warm-keepers bridge the final reciprocal chain (PE HAM clock)
        for _ in range(10):
            sdum = psum.tile([128, 1024], f32, tag="s")
            nc.tensor.matmul(
                sdum[0:65, 0:512], lhsT=vaug_sb[:, 0, 0:65],
                rhs=qt_sb[:, 0, :], start=True, stop=True,
            )
        recip_quarter(3)

        # ---- output projection: out[tok, D] = o_norm @ wo.T
        # 8 held accumulators: 2 psum tiles (4 halves) + 2 opsum + 1 apsum
        # tile (2 halves); wo streamed in [128, 2fc, 1024] chunks.
        for half in range(2):
            accs = []
            for i in range(2):
                s_t = psum.tile([128, 1024], f32, tag="s", name=f"oacc_s{half}_{i}")
                accs.append(s_t[:, 0:512])
                accs.append(s_t[:, 512:1024])
            for i in range(2):
                accs.append(opsum.tile([128, 512], f32, tag="o",
                                       name=f"oacc_o{half}_{i}"))
            po_t = apsum.tile([128, 1024], f32, tag="po", name=f"oacc_p{half}")
            accs.append(po_t[:, 0:512])
            accs.append(po_t[:, 512:1024])
            # accs[tb2*2+dc2]
            for fcg in range(8):
                wo_ch = wop.tile([128, 2, 1024], bf16, tag="woc")
                nc.sync.dma_start(
                    out=wo_ch[:],
                    in_=wo[:, 2 * fcg:2 * fcg + 2, 1024 * half:1024 * (half + 1)])
                for fl in range(2):
                    fc = 2 * fcg + fl
                    for tb2 in range(4):
                        tsl = slice(tb2 * 128, (tb2 + 1) * 128)
                        for dc2 in range(2):
                            nc.tensor.matmul(
                                accs[tb2 * 2 + dc2],
                                lhsT=ot_sb[:, fc, tsl],
                                rhs=wo_ch[:, fl, 512 * dc2:512 * (dc2 + 1)],
                                start=(fc == 0), stop=(fc == DC - 1),
                            )
            for tb2 in range(4):
                tsl = slice(tb2 * 128, (tb2 + 1) * 128)
                for dc2 in range(2):
                    osb = outw.tile([128, 512], f32, tag="osb")
                    nc.vector.tensor_copy(osb, accs[tb2 * 2 + dc2])
                    dsl = slice(1024 * half + 512 * dc2,
                                1024 * half + 512 * (dc2 + 1))
                    nc.sync.dma_start(out=out[tsl, dsl], in_=osb)

    nc.compile()
    return nc


def _prep_shared(freqs_cos, freqs_sin, wqkv, wo):
    """Weight/table prep shared by all cores."""
    cs = np.asarray(freqs_cos)[:, 0, :]  # [S, 64] (already repeat-2 layout)
    sn = np.asarray(freqs_sin)[:, 0, :]
    cos_h = np.empty((128, S), np.float32)
    sin_h = np.empty((128, S), np.float32)
    for p in range(128):
        cos_h[p] = cs[:, p % 64]
        sin_h[p] = sn[:, p % 64] * (-1.0 if p % 2 == 0 else 1.0)

    # Q rows permuted: fc = 4t+r -> [head 8t+r | head 8t+4+r]
    qrows = []
    for t in range(4):
        for r in range(4):
            for h in (8 * t + r, 8 * t + 4 + r):
                qrows.extend(range(h * HD, (h + 1) * HD))
    wq_t = np.ascontiguousarray(wqkv[qrows, :].T)  # [D, 2048]
    wq_h = np.ascontiguousarray(
        wq_t.reshape(DC, 128, DC, 128).transpose(1, 2, 0, 3)).astype(BF16)

    # K rows: tile t holds groups (2t | 2t+1)
    krows = []
    for t in range(4):
        for g in (2 * t, 2 * t + 1):
            krows.extend(range(H * HD + g * HD, H * HD + (g + 1) * HD))
    wk_t = np.ascontiguousarray(wqkv[krows, :].T)  # [D, 512]
    wk_h = np.ascontiguousarray(
        wk_t.reshape(DC, 128, 4, 128).transpose(1, 0, 2, 3)).astype(BF16)

    # V rows natural group order (cols t*128 : A 64 | B 64)
    vrows = list(range((H + G) * HD, (H + 2 * G) * HD))
    wv_t = np.ascontiguousarray(wqkv[vrows, :].T)  # [D, 512]
    wv_h = np.ascontiguousarray(
        wv_t.reshape(DC, 128, 512).transpose(1, 0, 2)).astype(BF16)

    # wo rhs: wo_h[p, fc, dcol] = wo[dcol, feat(fc, p)]
    feat = np.empty(D, np.int64)
    for fc in range(DC):
        t, r = divmod(fc, 4)
        for p in range(128):
            h = 8 * t + r + (4 if p >= 64 else 0)
            feat[fc * 128 + p] = h * HD + (p % 64)
    wo_h = np.ascontiguousarray(
        np.asarray(wo)[:, feat].T.reshape(DC, 128, D).transpose(1, 0, 2)
    ).astype(BF16)
    return cos_h, sin_h, wq_h, wk_h, wv_h, wo_h


def _prep_inputs(x, freqs_cos, freqs_sin, wqkv, wo):
    cos_h, sin_h, wq_h, wk_h, wv_h, wo_h = _prep_shared(
        freqs_cos, freqs_sin, wqkv, wo)
    x = np.asarray(x)
    cosf_h = np.ascontiguousarray(cos_h).astype(BF16)
    sinf_h = np.ascontiguousarray(sin_h).astype(BF16)
    xtf_hs = [
        np.ascontiguousarray(
            x[b].T.reshape(DC, 128, S).transpose(1, 0, 2)).astype(BF16)
        for b in range(B)
    ]
    ins = []
    for c in range(N_CORES):
        b, t4 = divmod(c, 4)
        q0 = t4 * TOK
        sl = slice(q0, q0 + TOK)
        xt_h = np.ascontiguousarray(xtf_hs[b][:, :, sl])
        ins.append({
            "xt": xt_h, "xtf": xtf_hs[b],
            "wq": wq_h, "wk": wk_h, "wv": wv_h, "wo": wo_h,
            "coso": np.ascontiguousarray(cos_h[:, sl]).astype(BF16),
            "sino": np.ascontiguousarray(sin_h[:, sl]).astype(BF16),
            "cosf": cosf_h, "sinf": sinf_h,
        })
    return ins


TRACE = False


def kernel(x, freqs_cos, freqs_sin, wqkv, wo):
    if "nc" not in _CACHE:
        _CACHE["nc"] = _build()
    nc = _CACHE["nc"]
    ins = _prep_inputs(x, freqs_cos, freqs_sin, wqkv, wo)
    res = run_bass_kernel_spmd(nc, ins, list(range(N_CORES)), trace=TRACE)
    _CACHE["res"] = res
    out = np.empty((B, S, D), np.float32)
    for c in range(N_CORES):
        b, t4 = divmod(c, 4)
        out[b, t4 * TOK:(t4 + 1) * TOK, :] = res.results[c]["out"]
    return out


if __name__ == "__main__":
    rng = np.random.default_rng(0)
    x = rng.normal(size=(B, S, D)).astype(np.float32)
    fc_ = rng.random(size=(S, 1, HD)).astype(np.float32)
    fs_ = rng.random(size=(S, 1, HD)).astype(np.float32)
    wq_ = rng.normal(size=(3072, D)).astype(np.float32) * 0.02
    wo_ = rng.normal(size=(D, D)).astype(np.float32) * 0.02
    o = kernel(x, fc_, fs_, wq_, wo_)
    print(o.shape, o.dtype)
